# revision 1
# baseline (speedup 1.0000x reference)
"""Trainium2 Bass kernel for nn_Decoder_24764781429449 (GNN message passing).

Math (per layer l of 3, N=4096 nodes, K=48 neighbors, C=128 channels, H=512):
    base   = concat([node0, zeros, edge])                  # fixed context
    mlp_in = concat([x, base])                             # [N,K,512]
    h1  = gelu(mlp_in @ W1 + b1)
    h2  = gelu(h1 @ W2 + b2)
    msg = h2 @ W3 + b3
    x   = LN1(x + sum_k(msg)/30);  x = LN2(x + dense_mlp(x));  x *= mask

Key algebraic reductions used here:
  * W1 rows 256:384 multiply zeros -> dead.
  * The x/node0 parts of the concat are shared across all K neighbors:
    h1 = gelu(edge @ W1d + pernode),  pernode = x@W1a + node0@W1b + b1.
  * sum_k (h2 @ W3 + b3) = (sum_k h2) @ W3 + K*b3; the k-sum is done by
    PSUM accumulation of per-k W3 matmuls.

Distribution: data-parallel over nodes, 512 nodes per core across 8 cores.
Edge features stay SBUF-resident (12.6MB/core), read from HBM exactly once.
All big matmuls run in float32r (TF32-like, full PE rate).
LayerNorm rsqrt is computed on the Vector engine with a bit-hack seed +
Newton iterations, so the Scalar engine runs Gelu only (no table switches).

Layout on device (per core):
  edge_km [128c, 24576]: col = half*12288 + k*256 + n  (half in {0,1}, n in 0..255)
  x / pernode: channel-major [128c, 512n]
  LayerNorm runs row-major via PE transposes: tiles [128n, 128c].
"""
import os
import numpy as np
from contextlib import ExitStack

import concourse.bass as bass
import concourse.bacc as bacc
import concourse.tile as tile
from concourse import mybir
from concourse.bass_utils import run_bass_kernel_spmd

F32 = mybir.dt.float32
F32R = mybir.dt.float32r
I32 = mybir.dt.int32
AF = mybir.ActivationFunctionType
OP = mybir.AluOpType

N, K, C, E, H, L = 4096, 48, 128, 128, 512, 3
NCORES = 8
NLOC = N // NCORES          # 512 nodes per core
NHALF = NLOC // 2           # 256
KPQ = 4                     # k-values per span
SPAN = KPQ * NHALF          # 1024 columns per span
NSPAN = K // KPQ            # 12 spans per half
SCALE = 30.0
EPS = 1e-5
MAGIC = 0x5F3759DF
NRM_T = NLOC // 128         # 4 row-major tiles of 128 nodes

_CACHED = {}
DBG = True


def _build():
    V_LAYERS = int(os.environ.get("KV_LAYERS", L))
    V_NOTAIL = os.environ.get("KV_NOTAIL", "0") == "1"
    V_REPS = int(os.environ.get("KV_REPS", "1"))
    V_ONECHUNK = os.environ.get("KV_ONECHUNK", "0") == "1"
    nc = bacc.Bacc()

    # ---------------- DRAM tensors ----------------
    edge_d = nc.dram_tensor("edge_km", [C, 2 * K * NHALF], F32R, kind="ExternalInput")
    x0_d = nc.dram_tensor("x0_ch", [C, NLOC], F32R, kind="ExternalInput")
    i128_d = nc.dram_tensor("i128", [C, C], F32R, kind="ExternalInput")
    w1a_d = nc.dram_tensor("w1a", [L, C, C], F32R, kind="ExternalInput")   # w1a[0] pre-folded with w1b[0]
    w1b_d = nc.dram_tensor("w1b", [L, C, C], F32R, kind="ExternalInput")
    w1d_d = nc.dram_tensor("w1d", [L, C, C], F32R, kind="ExternalInput")
    w2_d = nc.dram_tensor("w2", [L, C, C], F32R, kind="ExternalInput")
    w3e_d = nc.dram_tensor("w3e", [L, C, C], F32R, kind="ExternalInput")   # w3/SCALE
    dw1_d = nc.dram_tensor("dw1", [L, C, H], F32R, kind="ExternalInput")
    dw2_d = nc.dram_tensor("dw2", [L, C, 4 * C], F32R, kind="ExternalInput")
    # per-channel vectors: [L, C] each; db1 is [L, H] -> [L, 4, C]
    b1_d = nc.dram_tensor("b1", [L, C], F32, kind="ExternalInput")
    b2_d = nc.dram_tensor("b2", [L, C], F32, kind="ExternalInput")
    b3e_d = nc.dram_tensor("b3e", [L, C], F32, kind="ExternalInput")       # b3*K/SCALE
    db1_d = nc.dram_tensor("db1", [L, 4, C], F32, kind="ExternalInput")
    db2_d = nc.dram_tensor("db2", [L, C], F32, kind="ExternalInput")
    g1_d = nc.dram_tensor("ln1g", [L, C], F32, kind="ExternalInput")
    bb1_d = nc.dram_tensor("ln1b", [L, C], F32, kind="ExternalInput")
    g2_d = nc.dram_tensor("ln2g", [L, C], F32, kind="ExternalInput")
    bb2_d = nc.dram_tensor("ln2b", [L, C], F32, kind="ExternalInput")
    mask_d = nc.dram_tensor("mask_rm", [C, NRM_T], F32, kind="ExternalInput")
    out_d = nc.dram_tensor("out", [NLOC, C], F32, kind="ExternalOutput")
    if DBG:
        dbg_d = {
            "dbg_pern0": nc.dram_tensor("dbg_pern0", [C, NLOC], F32, kind="ExternalOutput"),
            "dbg_h1": nc.dram_tensor("dbg_h1", [C, SPAN], F32, kind="ExternalOutput"),
            "dbg_h2": nc.dram_tensor("dbg_h2", [C, SPAN], F32, kind="ExternalOutput"),
            "dbg_x1": nc.dram_tensor("dbg_x1", [C, NHALF], F32, kind="ExternalOutput"),
            "dbg_mv": nc.dram_tensor("dbg_mv", [C, 4], F32, kind="ExternalOutput"),
            "dbg_isd": nc.dram_tensor("dbg_isd", [C, 2], F32, kind="ExternalOutput"),
            "dbg_xln1": nc.dram_tensor("dbg_xln1", [C, NHALF], F32, kind="ExternalOutput"),
            "dbg_dh": nc.dram_tensor("dbg_dh", [C, 4 * NHALF], F32, kind="ExternalOutput"),
            "dbg_x2": nc.dram_tensor("dbg_x2", [C, NHALF], F32, kind="ExternalOutput"),
            "dbg_x3rm": nc.dram_tensor("dbg_x3rm", [C, 2 * C], F32, kind="ExternalOutput"),
            "dbg_xs1": nc.dram_tensor("dbg_xs1", [C, NLOC], F32, kind="ExternalOutput"),
        }

    def bcast_row(dram_ap):
        """Partition-broadcast a [C]-vector DRAM AP to [128, C]."""
        return bass.AP(tensor=dram_ap.tensor, offset=dram_ap.offset,
                       ap=[[0, 128]] + list(dram_ap.ap))

    with tile.TileContext(nc) as tc, ExitStack() as ctx:
        const = ctx.enter_context(tc.tile_pool(name="const", bufs=1))
        h1p = ctx.enter_context(tc.tile_pool(name="h1p", bufs=2))
        h2p = ctx.enter_context(tc.tile_pool(name="h2p", bufs=2))
        tl = ctx.enter_context(tc.tile_pool(name="tl", bufs=2))
        spanps = ctx.enter_context(tc.tile_pool(name="spanps", bufs=3, space="PSUM"))
        msump = ctx.enter_context(tc.tile_pool(name="msump", bufs=2, space="PSUM"))
        tailps = msump

        # ---------------- persistent SBUF ----------------
        edge = const.tile([C, 2 * K * NHALF], F32R)
        x0 = const.tile([C, NLOC], F32R)
        i128 = const.tile([C, C], F32R)
        w1a = const.tile([C, L * C], F32R)
        w1b = const.tile([C, L * C], F32R)
        w1d = const.tile([C, L * C], F32R)
        w2 = const.tile([C, L * C], F32R)
        w3e = const.tile([C, L * C], F32R)
        dw1 = const.tile([C, L * H], F32R)
        dw2 = const.tile([C, L * 4 * C], F32R)
        b1c = const.tile([C, L], F32)
        b2c = const.tile([C, L], F32)
        b3ec = const.tile([C, L], F32)
        db1c = const.tile([C, L * 4], F32)
        db2c = const.tile([C, L], F32)
        gbc1 = const.tile([C, L * C], F32)   # row-major per-channel broadcast tiles
        bbc1 = const.tile([C, L * C], F32)
        gbc2 = const.tile([C, L * C], F32)
        bbc2 = const.tile([C, L * C], F32)
        maskc = const.tile([C, NRM_T], F32)
        magic = const.tile([C, 1], I32)
        n0pern = const.tile([C, 2 * NLOC], F32)      # l=1,2
        pern = [const.tile([C, NLOC], F32R, name=f"pern{l}", tag=f"pern{l}") for l in range(L)]
        xs = [x0] + [const.tile([C, NLOC], F32R, name=f"x{l}", tag=f"x{l}") for l in (1, 2)]

        nc.vector.memset(magic, MAGIC)

        # ---------------- input DMAs ----------------
        for l in range(L):
            nc.sync.dma_start(w1a[:, l * C:(l + 1) * C], w1a_d.ap()[l])
            nc.sync.dma_start(w1b[:, l * C:(l + 1) * C], w1b_d.ap()[l])
            nc.sync.dma_start(w1d[:, l * C:(l + 1) * C], w1d_d.ap()[l])
            nc.sync.dma_start(w2[:, l * C:(l + 1) * C], w2_d.ap()[l])
            nc.sync.dma_start(w3e[:, l * C:(l + 1) * C], w3e_d.ap()[l])
            nc.sync.dma_start(dw1[:, l * H:(l + 1) * H], dw1_d.ap()[l])
            nc.sync.dma_start(dw2[:, l * 4 * C:(l + 1) * 4 * C], dw2_d.ap()[l])
            nc.sync.dma_start(b1c[:, l:l + 1], b1_d.ap()[l].rearrange("(c one) -> c one", one=1))
            nc.sync.dma_start(b2c[:, l:l + 1], b2_d.ap()[l].rearrange("(c one) -> c one", one=1))
            nc.sync.dma_start(b3ec[:, l:l + 1], b3e_d.ap()[l].rearrange("(c one) -> c one", one=1))
            nc.sync.dma_start(db2c[:, l:l + 1], db2_d.ap()[l].rearrange("(c one) -> c one", one=1))
            for hh in range(4):
                nc.sync.dma_start(db1c[:, l * 4 + hh:l * 4 + hh + 1],
                                  db1_d.ap()[l, hh].rearrange("(c one) -> c one", one=1))
            nc.sync.dma_start(gbc1[:, l * C:(l + 1) * C], bcast_row(g1_d.ap()[l]))
            nc.sync.dma_start(bbc1[:, l * C:(l + 1) * C], bcast_row(bb1_d.ap()[l]))
            nc.sync.dma_start(gbc2[:, l * C:(l + 1) * C], bcast_row(g2_d.ap()[l]))
            nc.sync.dma_start(bbc2[:, l * C:(l + 1) * C], bcast_row(bb2_d.ap()[l]))
        nc.sync.dma_start(i128, i128_d.ap())
        nc.sync.dma_start(maskc, mask_d.ap())
        nc.sync.dma_start(x0, x0_d.ap())
        # edge chunks: fine-grained, spread across HW and SW DMA queues
        ECH = NHALF  # 256-col chunks
        nchunks = 1 if V_ONECHUNK else (2 * K * NHALF // ECH)
        for cchunk in range(nchunks):
            sl = slice(cchunk * ECH, (cchunk + 1) * ECH)
            eng = nc.sync if cchunk % 2 == 0 else nc.gpsimd
            eng.dma_start(edge[:, sl], edge_d.ap()[:, sl])

        # ---------------- setup: n0pern (l=1,2), pern[0] ----------------
        for li, l in enumerate((1, 2)):
            pp = tailps.tile([C, NLOC], F32, tag="ps1", name="pp")
            nc.tensor.matmul(pp, w1b[:, l * C:(l + 1) * C], x0, start=True, stop=True)
            nc.vector.tensor_copy(n0pern[:, li * NLOC:(li + 1) * NLOC], pp)
        pp = tailps.tile([C, NLOC], F32, tag="ps1", name="pp")
        nc.tensor.matmul(pp, w1a[:, 0:C], x0, start=True, stop=True)
        # pern0 = pp + b1[0]  (w1a[0] holds w1a+w1b pre-folded)
        nc.vector.tensor_scalar(pern[0], pp, b1c[:, 0:1], None, op0=OP.add)

        if DBG:
            nc.sync.dma_start(dbg_d["dbg_pern0"].ap(), pern[0].bitcast(F32))

        # quake rsqrt helper: writes 1/sqrt(v+EPS) into `dst` ([128, n] fp32)
        def quake_rsqrt(pool_tile_cols, var_ap, dst):
            n = pool_tile_cols
            veps = tl.tile([C, n], F32, tag="q_veps")
            nc.vector.tensor_scalar(veps, var_ap, EPS, None, op0=OP.add)
            ush = tl.tile([C, n], I32, tag="q_ush")
            nc.vector.tensor_scalar(ush, veps.bitcast(I32), 1, None,
                                    op0=OP.logical_shift_right)
            nc.vector.scalar_tensor_tensor(
                dst.bitcast(I32), in0=magic.broadcast_to([C, n]), scalar=0,
                in1=ush, op0=OP.bypass, op1=OP.subtract)
            t = tl.tile([C, n], F32, tag="q_t")
            for _ in range(3):
                nc.vector.tensor_mul(t, dst, dst)
                nc.vector.tensor_mul(t, t, veps)
                nc.vector.tensor_scalar(t, t, -0.5, 1.5, op0=OP.mult, op1=OP.add)
                nc.vector.tensor_mul(dst, dst, t)

        # ---------------- main: layers x halves ----------------
        for _rep in range(V_REPS):
          for l in range(V_LAYERS):
            for h in range(2):
                nsl = slice(h * NHALF, (h + 1) * NHALF)
                msum = msump.tile([C, NHALF], F32, tag="ps1", name="msum")
                # ---- message-MLP spans ----
                # Software-pipelined spans: gelu-A of span s+1 is emitted
                # before gelu-B of span s so the ACT stream never waits on
                # the W2 matmuls; msum matmuls trail one more step.
                h1s, t2s, h2s = {}, {}, {}

                def emit_mm1(s):
                    t1 = spanps.tile([C, SPAN], F32, tag="span", name="t1")
                    for q in range(KPQ):
                        rsl = slice(q * NHALF, (q + 1) * NHALF)
                        col0 = 0 if V_ONECHUNK else (h * (K * NHALF) + (s * KPQ + q) * NHALF)
                        nc.tensor.matmul(t1[:, rsl], i128, pern[l][:, nsl],
                                         start=True, stop=False)
                        nc.tensor.matmul(t1[:, rsl], w1d[:, l * C:(l + 1) * C],
                                         edge[:, col0:col0 + NHALF],
                                         start=False, stop=True)
                    return t1

                def emit_geluA(s, t1):
                    h1 = h1p.tile([C, SPAN], F32R, tag="h1", name="h1")
                    nc.scalar.activation(h1, t1, AF.Gelu)
                    if DBG and l == 0 and h == 0 and s == 0:
                        nc.sync.dma_start(dbg_d["dbg_h1"].ap(), h1.bitcast(F32))
                    h1s[s] = h1

                def emit_B(s):
                    h1 = h1s.pop(s)
                    t2 = spanps.tile([C, SPAN], F32, tag="span", name="t2")
                    for j in range(2):
                        jsl = slice(j * 512, (j + 1) * 512)
                        nc.tensor.matmul(t2[:, jsl], w2[:, l * C:(l + 1) * C],
                                         h1[:, jsl], start=True, stop=True)
                    h2 = h2p.tile([C, SPAN], F32R, tag="h2", name="h2")
                    nc.scalar.activation(h2, t2, AF.Gelu, bias=b2c[:, l:l + 1])
                    if DBG and l == 0 and h == 0 and s == 0:
                        nc.sync.dma_start(dbg_d["dbg_h2"].ap(), h2.bitcast(F32))
                    h2s[s] = h2

                def emit_msum(s):
                    h2 = h2s.pop(s)
                    for q in range(KPQ):
                        rsl = slice(q * NHALF, (q + 1) * NHALF)
                        nc.tensor.matmul(msum, w3e[:, l * C:(l + 1) * C], h2[:, rsl],
                                         start=(s == 0 and q == 0),
                                         stop=(s == NSPAN - 1 and q == KPQ - 1))

                for s in range(NSPAN):
                    t1 = emit_mm1(s)
                    emit_geluA(s, t1)
                    if s >= 1:
                        emit_B(s - 1)
                    if s >= 2:
                        emit_msum(s - 2)
                emit_B(NSPAN - 1)
                emit_msum(NSPAN - 2)
                emit_msum(NSPAN - 1)
                # ---- node tail for this half ----
                # x1 = x + msum + b3e
                x1 = tl.tile([C, NHALF], F32, tag="x1")
                nc.vector.scalar_tensor_tensor(
                    x1, in0=msum, scalar=b3ec[:, l:l + 1], in1=xs[l].bitcast(F32)[:, nsl],
                    op0=OP.add, op1=OP.add)
                if DBG and l == 0 and h == 0:
                    nc.sync.dma_start(dbg_d["dbg_x1"].ap(), x1)
                # transpose to row-major
                x1rm = tl.tile([C, 2, C], F32, tag="x1rm")
                for t in range(2):
                    tp = tailps.tile([C, C], F32, tag="ps1", name="tp")
                    nc.tensor.transpose(tp, x1[:, t * C:(t + 1) * C], i128.bitcast(F32))
                    nc.vector.tensor_copy(x1rm[:, t], tp)
                # LN1 stats
                st = tl.tile([C, 2, 6], F32, tag="st")
                mv = tl.tile([C, 2, 2], F32, tag="mv")
                for t in range(2):
                    nc.vector.bn_stats(st[:, t], x1rm[:, t])
                    nc.vector.bn_aggr(mv[:, t], st[:, t])
                isd = tl.tile([C, 2], F32, tag="isd")
                quake_rsqrt(2, mv[:, :, 1], isd)
                if DBG and l == 0 and h == 0:
                    nc.sync.dma_start(dbg_d["dbg_mv"].ap(), mv.rearrange("p a b -> p (a b)"))
                    nc.sync.dma_start(dbg_d["dbg_isd"].ap(), isd)
                # apply LN1: xln = ((x1rm - mu) * is) * g + b
                xln1rm = tl.tile([C, 2, C], F32, tag="xln1rm")
                for t in range(2):
                    nc.vector.tensor_scalar(xln1rm[:, t], x1rm[:, t],
                                            mv[:, t, 0:1], isd[:, t:t + 1],
                                            op0=OP.subtract, op1=OP.mult)
                    nc.vector.tensor_mul(xln1rm[:, t], xln1rm[:, t], gbc1[:, l * C:(l + 1) * C])
                    nc.vector.tensor_add(xln1rm[:, t], xln1rm[:, t], bbc1[:, l * C:(l + 1) * C])
                # transpose back to channel-major
                xln1 = tl.tile([C, NHALF], F32R, tag="xln1")
                for t in range(2):
                    tp = tailps.tile([C, C], F32, tag="ps1", name="tp")
                    nc.tensor.transpose(tp, xln1rm[:, t], i128.bitcast(F32))
                    nc.vector.tensor_copy(xln1[:, t * C:(t + 1) * C], tp)
                if DBG and l == 0 and h == 0:
                    nc.sync.dma_start(dbg_d["dbg_xln1"].ap(), xln1.bitcast(F32))
                # dense MLP
                dh = tl.tile([C, 4 * NHALF], F32R, tag="dh")
                for hh in range(4):
                    pd = tailps.tile([C, NHALF], F32, tag="ps1", name="pd")
                    nc.tensor.matmul(pd, dw1[:, l * H + hh * C:l * H + (hh + 1) * C],
                                     xln1, start=True, stop=True)
                    nc.scalar.activation(dh[:, hh * NHALF:(hh + 1) * NHALF], pd,
                                         AF.Gelu, bias=db1c[:, l * 4 + hh:l * 4 + hh + 1])
                if DBG and l == 0 and h == 0:
                    nc.sync.dma_start(dbg_d["dbg_dh"].ap(), dh.bitcast(F32))
                pd2 = tailps.tile([C, NHALF], F32, tag="ps1", name="pd2")
                for hh in range(4):
                    nc.tensor.matmul(pd2, dw2[:, (l * 4 + hh) * C:(l * 4 + hh + 1) * C],
                                     dh[:, hh * NHALF:(hh + 1) * NHALF],
                                     start=(hh == 0), stop=(hh == 3))
                # x2 = xln1 + dense + db2
                x2 = tl.tile([C, NHALF], F32, tag="x2")
                nc.vector.scalar_tensor_tensor(
                    x2, in0=pd2, scalar=db2c[:, l:l + 1], in1=xln1.bitcast(F32),
                    op0=OP.add, op1=OP.add)
                if DBG and l == 0 and h == 0:
                    nc.sync.dma_start(dbg_d["dbg_x2"].ap(), x2)
                # transpose to row-major
                x2rm = tl.tile([C, 2, C], F32, tag="x2rm")
                for t in range(2):
                    tp = tailps.tile([C, C], F32, tag="ps1", name="tp")
                    nc.tensor.transpose(tp, x2[:, t * C:(t + 1) * C], i128.bitcast(F32))
                    nc.vector.tensor_copy(x2rm[:, t], tp)
                # LN2
                st2 = tl.tile([C, 2, 6], F32, tag="st")
                mv2 = tl.tile([C, 2, 2], F32, tag="mv")
                for t in range(2):
                    nc.vector.bn_stats(st2[:, t], x2rm[:, t])
                    nc.vector.bn_aggr(mv2[:, t], st2[:, t])
                isd2 = tl.tile([C, 2], F32, tag="isd")
                quake_rsqrt(2, mv2[:, :, 1], isd2)
                x3rm = tl.tile([C, 2, C], F32, tag="x3rm")
                for t in range(2):
                    nc.vector.tensor_scalar(x3rm[:, t], x2rm[:, t],
                                            mv2[:, t, 0:1], isd2[:, t:t + 1],
                                            op0=OP.subtract, op1=OP.mult)
                    nc.vector.tensor_mul(x3rm[:, t], x3rm[:, t], gbc2[:, l * C:(l + 1) * C])
                    nc.vector.tensor_add(x3rm[:, t], x3rm[:, t], bbc2[:, l * C:(l + 1) * C])
                    # mask (per-node = per-partition in row-major)
                    nc.vector.tensor_scalar(x3rm[:, t], x3rm[:, t],
                                            maskc[:, 2 * h + t:2 * h + t + 1], None,
                                            op0=OP.mult)
                if DBG and l == 0 and h == 0:
                    nc.sync.dma_start(dbg_d["dbg_x3rm"].ap(), x3rm.rearrange("p a b -> p (a b)"))
                if l < L - 1:
                    # transpose back into xs[l+1] and compute pern[l+1] slice
                    for t in range(2):
                        tp = tailps.tile([C, C], F32, tag="ps1", name="tp")
                        nc.tensor.transpose(tp, x3rm[:, t], i128.bitcast(F32))
                        nc.vector.tensor_copy(
                            xs[l + 1][:, h * NHALF + t * C:
                                      h * NHALF + (t + 1) * C], tp)
                    pp = tailps.tile([C, NHALF], F32, tag="ps1", name="pp")
                    nc.tensor.matmul(pp, w1a[:, (l + 1) * C:(l + 2) * C], xs[l + 1][:, nsl],
                                     start=True, stop=True)
                    nc.vector.scalar_tensor_tensor(
                        pern[l + 1][:, nsl], in0=pp,
                        scalar=b1c[:, l + 1:l + 2], in1=n0pern[:, l * NLOC + h * NHALF:l * NLOC + (h + 1) * NHALF],
                        op0=OP.add, op1=OP.add)
                else:
                    for t in range(2):
                        nc.sync.dma_start(
                            out_d.ap()[h * NHALF + t * C:h * NHALF + (t + 1) * C, :],
                            x3rm[:, t])

        if DBG:
            nc.sync.dma_start(dbg_d["dbg_xs1"].ap(), xs[1].bitcast(F32))

    nc.compile()
    return nc


def _prep_inputs(inputs):
    """Host-side: shard over nodes, relayout, fold weight-only arithmetic."""
    nf = np.asarray(inputs["node_features"], dtype=np.float32)
    ef = np.asarray(inputs["edge_features"], dtype=np.float32)
    mask = np.asarray(inputs["mask"], dtype=np.float32)
    w1 = np.asarray(inputs["msg_w1"], dtype=np.float32)
    w2 = np.asarray(inputs["msg_w2"], dtype=np.float32)
    w3 = np.asarray(inputs["msg_w3"], dtype=np.float32)

    w1a = w1[:, 0:C, :].copy()
    w1b = w1[:, C:2 * C, :].copy()
    w1d = w1[:, 3 * C:4 * C, :].copy()
    # layer 0: x == node0, fold both contributions into w1a[0]
    w1a[0] = w1a[0] + w1b[0]
    w3e = (w3 / SCALE).copy()
    b3e = (np.asarray(inputs["msg_b3"], dtype=np.float32) * (K / SCALE)).copy()
    dw2 = np.ascontiguousarray(np.asarray(inputs["d_w2"], dtype=np.float32)
                               .reshape(L, 4, C, C).transpose(0, 2, 1, 3)
                               .reshape(L, C, 4 * C))
    db1 = np.asarray(inputs["d_b1"], dtype=np.float32).reshape(L, 4, C).copy()

    shared = {
        "i128": np.eye(C, dtype=np.float32),
        "w1a": w1a, "w1b": w1b, "w1d": w1d, "w2": w2.copy(), "w3e": w3e,
        "dw1": np.asarray(inputs["d_w1"], dtype=np.float32).copy(),
        "dw2": dw2,
        "b1": np.asarray(inputs["msg_b1"], dtype=np.float32).copy(),
        "b2": np.asarray(inputs["msg_b2"], dtype=np.float32).copy(),
        "b3e": b3e, "db1": db1,
        "db2": np.asarray(inputs["d_b2"], dtype=np.float32).copy(),
        "ln1g": np.asarray(inputs["ln1_g"], dtype=np.float32).copy(),
        "ln1b": np.asarray(inputs["ln1_b"], dtype=np.float32).copy(),
        "ln2g": np.asarray(inputs["ln2_g"], dtype=np.float32).copy(),
        "ln2b": np.asarray(inputs["ln2_b"], dtype=np.float32).copy(),
    }

    in_maps = []
    for core in range(NCORES):
        n0 = core * NLOC
        esh = ef[n0:n0 + NLOC]                       # [512, 48, 128]
        ekm = esh.transpose(2, 1, 0)                 # [128c, 48k, 512n]
        ekm = ekm.reshape(C, K, 2, NHALF).transpose(0, 2, 1, 3)  # [c, half, k, n]
        ekm = np.ascontiguousarray(ekm.reshape(C, 2 * K * NHALF))
        msh = mask[n0:n0 + NLOC]
        mask_rm = np.ascontiguousarray(msh.reshape(NRM_T, C).T)  # [128, 4]
        m = dict(shared)
        m["edge_km"] = ekm
        m["x0_ch"] = np.ascontiguousarray(nf[n0:n0 + NLOC].T)
        m["mask_rm"] = mask_rm
        in_maps.append(m)
    return in_maps


def kernel(**inputs) -> np.ndarray:
    if "nc" not in _CACHED:
        _CACHED["nc"] = _build()
    nc = _CACHED["nc"]
    in_maps = _prep_inputs(inputs)
    res = run_bass_kernel_spmd(nc, in_maps, core_ids=list(range(NCORES)))
    out = np.concatenate([res.results[c]["out"] for c in range(NCORES)], axis=0)
    _CACHED["last_results"] = res
    return out

